# revision 1
# baseline (speedup 1.0000x reference)
import os
import sys

sys.path.insert(0, '/opt/trn_rl_repo')
import numpy as np

NCORES = 8
N = 100000
H = 128
GROUPS = 4
GC = H // GROUPS
K = 5
SHARD_N = N // NCORES          # 12500 owned real nodes per core
TILES = 98
SHARD = TILES * 128            # 12544 padded
FULL = SHARD * NCORES          # 100352
WIN = 2 * SHARD                # 25088 rows per gather window (int16-safe)
NWIN = 4
SC = 7                         # tiles per super-chunk
NCHUNK = TILES // SC           # 14
RMS_EPS = 1.1920929e-07

_cache = {}


def _build_and_run(x_full, x_own_all, idxw_all, ew_all, params_all, rmsw_all,
                   schedule, repeat=1):
    from concourse import bacc, bass, mybir, tile
    from concourse.bass_utils import run_bass_kernel_spmd

    (R_tw, call_rounds, col0, TR, chunks) = schedule
    f32 = mybir.dt.float32
    i16 = mybir.dt.int16

    key = ("prog", TR, os.environ.get("KERNEL_ABLATE"),
           tuple(call_rounds.ravel()),
           tuple(tuple(ts) for ts in chunks), repeat)
    if key in _cache:
        nc = _cache[key]
    else:
        nc = bacc.Bacc("TRN2", target_bir_lowering=False, debug=False,
                       num_devices=NCORES)
        xf = nc.dram_tensor("x_full", [FULL, H], f32, kind="ExternalInput")
        xo = nc.dram_tensor("x_own", [SHARD, H], f32, kind="ExternalInput")
        idx_in = nc.dram_tensor("idx_in", [128, 8 * TR], i16, kind="ExternalInput")
        ew_in = nc.dram_tensor("ew_in", [128, TR], f32, kind="ExternalInput")
        par_in = nc.dram_tensor("par_in", [128, 40], f32, kind="ExternalInput")
        rw_in = nc.dram_tensor("rw_in", [128, H], f32, kind="ExternalInput")
        out_ext = nc.dram_tensor("out", [SHARD, H], f32, kind="ExternalOutput")

        with tile.TileContext(nc) as tc:
            with (
                tc.tile_pool(name="dram", bufs=1, space="DRAM") as dram,
                tc.tile_pool(name="big", bufs=1) as big,
                tc.tile_pool(name="stage", bufs=3) as stage,
                tc.tile_pool(name="small", bufs=2) as small,
            ):
                shard_b = [dram.tile([SHARD, H], f32, tag=f"sh{k}",
                                     name=f"sh{k}") for k in range(5)]
                full_b = [dram.tile([FULL, H], f32, tag=f"fl{k}",
                                    name=f"fl{k}", addr_space="Shared")
                          for k in range(4)]

                idx_sb = big.tile([128, 8 * TR], i16, tag="idx")
                ew_sb = big.tile([128, TR], f32, tag="ew")
                par_sb = big.tile([128, 40], f32, tag="par")
                rw_sb = big.tile([128, H], f32, tag="rw")
                nc.sync.dma_start(idx_sb[:], idx_in[:])
                nc.sync.dma_start(ew_sb[:], ew_in[:])
                nc.sync.dma_start(par_sb[:], par_in[:])
                nc.sync.dma_start(rw_sb[:], rw_in[:])

                result = big.tile([128, SHARD], f32, tag="res")
                tcur = big.tile([128, SHARD], f32, tag="tcur")

                def res3(ap):  # [128, SHARD] -> [p, t, j]
                    return ap.rearrange("p (t j) -> p t j", j=H)

                def gview(ap, g):  # group-g strided view [p, t, 32]
                    return res3(ap)[:, :, g * GC:(g + 1) * GC]

                def cheb(g, k):
                    return par_sb[:, g * 6 + k:g * 6 + k + 1]

                TRS_MAX = int(call_rounds.sum(axis=1).max())
                NT_MAX = max(len(ts) for ts in chunks)

                for rep in range(repeat):
                    if rep > 0:
                        full_b = [dram.tile([FULL, H], f32, tag=f"fl{k}_{rep}",
                                            name=f"fl{k}_{rep}",
                                            addr_space="Shared")
                                  for k in range(4)]
                    # result = x_own ; then scale by c0 per group
                    nc.sync.dma_start(
                        res3(result[:]),
                        xo[:].rearrange("(t p) j -> p t j", p=128))
                    for g in range(GROUPS):
                        nc.vector.tensor_mul(
                            gview(result[:], g), gview(result[:], g),
                            cheb(g, 0).to_broadcast([128, TILES, GC]))

                    for hop in range(1, K + 1):
                        if hop == 1:
                            src_tab = xf
                        else:
                            src_tab = full_b[hop - 2]
                        for s in range(len(chunks)):
                            ts = chunks[s]
                            nt = len(ts)
                            t0_, t1_ = ts[0], ts[-1] + 1
                            gbuf = stage.tile([128, TRS_MAX, H], f32, tag="g")
                            # gather calls, one per (chunk, window)
                            base = 0
                            for w in range(NWIN):
                                rw_ = int(call_rounds[s, w])
                                if rw_ == 0:
                                    continue
                                nidx = rw_ * 128
                                cb = col0[s, w]  # global round col of call start
                                if os.environ.get("KERNEL_ABLATE") != "gather":
                                    nc.gpsimd.dma_gather(
                                        gbuf[:, base:base + rw_, :],
                                        src_tab[w * WIN:(w + 1) * WIN, :],
                                        idx_sb[:, 8 * cb:8 * (cb + rw_)],
                                        nidx, nidx, H, single_packet=False,
                                    )
                                base += rw_
                            # Tprev2 chunk (hops >= 2)
                            if hop >= 2:
                                tp2 = small.tile([128, NT_MAX, H], f32, tag="tp2")
                                pb = shard_b[hop - 3][:] if hop >= 3 else xo[:]
                                nc.sync.dma_start(
                                    tp2[:, :nt, :],
                                    pb[t0_ * 128:t1_ * 128, :]
                                    .rearrange("(t p) j -> p t j", p=128))
                            # compute per tile
                            for tl in range(nt):
                                t = ts[tl]
                                dst = tcur[:, t * H:(t + 1) * H]
                                # slots of this tile inside gbuf + global cols
                                slots = schedule_tile_slots(R_tw, call_rounds,
                                                            col0, chunks, s, tl)
                                if hop >= 2:
                                    tgt = small.tile([128, H], f32, tag="prop")
                                else:
                                    tgt = None
                                ops_out = (tgt[:] if hop >= 2 else dst)
                                if os.environ.get("KERNEL_ABLATE") == "dve":
                                    slots = slots[:1]
                                if not slots:
                                    nc.vector.memset(ops_out, 0.0)
                                else:
                                    q0, c0_ = slots[0]
                                    nc.vector.tensor_mul(
                                        ops_out, gbuf[:, q0, :],
                                        ew_sb[:, c0_:c0_ + 1]
                                        .to_broadcast([128, H]))
                                    for (q, cc) in slots[1:]:
                                        nc.vector.tensor_mul(
                                            gbuf[:, q, :], gbuf[:, q, :],
                                            ew_sb[:, cc:cc + 1]
                                            .to_broadcast([128, H]))
                                        nc.vector.tensor_add(
                                            ops_out, ops_out, gbuf[:, q, :])
                                if hop >= 2:
                                    nc.vector.scalar_tensor_tensor(
                                        dst, tgt[:], 2.0, tp2[:, tl, :],
                                        mybir.AluOpType.mult,
                                        mybir.AluOpType.subtract)
                            # store Tcur chunk to shard bounce (not on hop 5)
                            if hop <= 4:
                                nc.sync.dma_start(
                                    shard_b[hop - 1]
                                    [t0_ * 128:t1_ * 128, :]
                                    .rearrange("(t p) j -> p t j", p=128),
                                    tcur[:].rearrange("p (t j) -> p t j", j=H)
                                    [:, t0_:t1_, :])
                        # result += c_hop * tcur (tcur scaled in place
                        # after its DMA stores; safe, it is dead afterwards)
                        for g in range(GROUPS):
                            nc.vector.tensor_mul(
                                gview(tcur[:], g), gview(tcur[:], g),
                                cheb(g, hop).to_broadcast([128, TILES, GC]))
                        nc.vector.tensor_add(result[:], result[:], tcur[:])
                        # AllGather
                        if hop <= 4:
                            nc.gpsimd.collective_compute(
                                "AllGather", mybir.AluOpType.bypass,
                                replica_groups=[list(range(NCORES))],
                                ins=[shard_b[hop - 1][:].opt()],
                                outs=[full_b[hop - 1][:].opt()],
                            )

                    # epilogue: group scale/bias, RMSNorm, SiLU
                    for g in range(GROUPS):
                        nc.vector.tensor_mul(
                            gview(result[:], g), gview(result[:], g),
                            par_sb[:, 24 + g:25 + g]
                            .to_broadcast([128, TILES, GC]))
                        nc.vector.tensor_add(
                            gview(result[:], g), gview(result[:], g),
                            par_sb[:, 28 + g:29 + g]
                            .to_broadcast([128, TILES, GC]))
                    # sumsq
                    nc.vector.tensor_mul(tcur[:], result[:], result[:])
                    ssq = small.tile([128, TILES], f32, tag="ssq")
                    nc.vector.tensor_reduce(
                        ssq[:], res3(tcur[:]), mybir.AxisListType.X,
                        mybir.AluOpType.add)
                    rms = small.tile([128, TILES], f32, tag="rms")
                    sq = small.tile([128, TILES], f32, tag="sqr")
                    nc.scalar.activation(
                        sq[:], ssq[:], mybir.ActivationFunctionType.Sqrt,
                        bias=par_sb[:, 32:33], scale=1.0 / H)
                    nc.vector.reciprocal(rms[:], sq[:])
                    # result *= rms (broadcast over j), *= rms_weight (bcast t)
                    nc.vector.tensor_mul(
                        res3(result[:]), res3(result[:]),
                        rms[:].rearrange("p (t o) -> p t o", o=1).to_broadcast(
                            [128, TILES, H]))
                    nc.vector.tensor_mul(
                        res3(result[:]), res3(result[:]),
                        rw_sb[:].rearrange("p (o j) -> p o j", o=1)
                        .to_broadcast([128, TILES, H]))
                    nc.scalar.activation(
                        res3(tcur[:]), res3(result[:]),
                        mybir.ActivationFunctionType.Silu)
                    nc.sync.dma_start(
                        out_ext[:].rearrange("(t p) j -> p t j", p=128),
                        res3(tcur[:]))

        nc.compile()
        _cache[key] = nc

    in_maps = []
    for c in range(NCORES):
        in_maps.append({
            "x_full": x_full,
            "x_own": x_own_all[c],
            "idx_in": idxw_all[c],
            "ew_in": ew_all[c],
            "par_in": params_all,
            "rw_in": rmsw_all,
        })
    res = run_bass_kernel_spmd(nc, in_maps, list(range(NCORES)))
    return np.stack([res.results[c]["out"] for c in range(NCORES)], axis=0)


def schedule_tile_slots(R_tw, call_rounds, col0, chunks, s, tl):
    """Return [(slot_in_gbuf, global_ew_col)] for chunks[s][tl], in round order."""
    ts = chunks[s]
    t = ts[tl]
    slots = []
    base = 0
    for w in range(NWIN):
        off = int(R_tw[np.array(ts[:tl]), w].sum()) if tl else 0
        for r in range(int(R_tw[t, w])):
            slot = base + off + r
            gcol = int(col0[s, w]) + off + r
            slots.append((slot, gcol))
        base += int(call_rounds[s, w])
    return slots


def _prep(x, edge_weight_norm, edge_index):
    src = np.asarray(edge_index[0]).astype(np.int64)
    dst = np.asarray(edge_index[1]).astype(np.int64)
    ew = np.asarray(edge_weight_norm, dtype=np.float32)
    E = src.shape[0]

    # window class of a node = v % NWIN (assignment-independent);
    # class-w nodes are dealt to cores {2w, 2w+1} so their padded positions
    # fill exactly window w = [w*WIN, (w+1)*WIN).
    prof = np.zeros((N, NWIN), np.int32)
    np.add.at(prof, (dst, src % NWIN), 1)

    perm_pos = np.empty(N, np.int64)
    inv = np.empty(FULL, np.int64)
    inv.fill(-1)
    CLS_N = N // NWIN  # 25000
    for w in range(NWIN):
        nodes = np.arange(N)[np.arange(N) % NWIN == w]
        kk = prof[nodes]
        order = np.lexsort((kk[:, 3], kk[:, 2], kk[:, 1], kk[:, 0]))
        sn = nodes[order]
        rank = np.arange(CLS_N)
        core = 2 * w + (rank % 2)
        pos = core * SHARD + rank // 2
        perm_pos[sn] = pos
        inv[pos] = sn

    x_full = np.zeros((FULL, H), np.float32)
    x_full[perm_pos] = np.asarray(x, np.float32)

    src_p = perm_pos[src]
    dst_p = perm_pos[dst]
    dst_core = dst_p // SHARD
    win_id = src_p // WIN
    win_rel = (src_p - win_id * WIN).astype(np.int64)
    dst_local = dst_p - dst_core * SHARD
    tile_id = dst_local // 128
    part_id = dst_local % 128

    # counts per (core, tile, part, window)
    cnt = np.zeros((NCORES, TILES, 128, NWIN), np.int32)
    np.add.at(cnt, (dst_core, tile_id, part_id, win_id), 1)
    R_tw = cnt.max(axis=(0, 2))  # [TILES, NWIN] shared across cores

    # greedy variable chunks: pack tiles until round budget CAP
    CAP = 44
    R_t = R_tw.sum(axis=1)
    chunks = []
    cur = []
    acc = 0
    for t in range(TILES):
        if cur and acc + R_t[t] > CAP:
            chunks.append(cur)
            cur = []
            acc = 0
        cur.append(t)
        acc += int(R_t[t])
    if cur:
        chunks.append(cur)
    nch = len(chunks)
    call_rounds = np.zeros((nch, NWIN), np.int64)
    for s, ts in enumerate(chunks):
        for w in range(NWIN):
            call_rounds[s, w] = R_tw[np.array(ts), w].sum()
    col0 = np.zeros((nch, NWIN), np.int64)
    run = 0
    for s in range(nch):
        for w in range(NWIN):
            col0[s, w] = run
            run += call_rounds[s, w]
    TR = int(run)

    chunk_of = np.zeros(TILES, np.int64)
    off_tw = np.zeros((TILES, NWIN), np.int64)
    for s, ts in enumerate(chunks):
        accw = np.zeros(NWIN, np.int64)
        for t in ts:
            chunk_of[t] = s
            off_tw[t] = accw
            accw += R_tw[t]
    # rank of edge within its (core, tile, part, window) group
    key = (((dst_core * TILES + tile_id) * 128 + part_id) * NWIN + win_id)
    order = np.argsort(key, kind='stable')
    ks = key[order]
    starts = np.r_[0, np.flatnonzero(np.diff(ks)) + 1]
    group_len = np.diff(np.r_[starts, E])
    rank_sorted = np.arange(E) - np.repeat(starts, group_len)
    rank = np.empty(E, np.int64)
    rank[order] = rank_sorted

    gcol = col0[chunk_of[tile_id], win_id] + off_tw[tile_id, win_id] + rank

    # fill ew and idx flats per core
    ew_all = [np.zeros((128, TR), np.float32) for _ in range(NCORES)]
    idx_flat = [np.zeros(TR * 128, np.int16) for _ in range(NCORES)]
    for c in range(NCORES):
        m = dst_core == c
        ew_all[c][part_id[m], gcol[m]] = ew[m]
        idx_flat[c][gcol[m] * 128 + part_id[m]] = win_rel[m].astype(np.int16)

    # wrapped idx layout per call, concatenated: col block [8*cb, 8*(cb+R))
    idxw_all = []
    for c in range(NCORES):
        iw = np.zeros((128, 8 * TR), np.int16)
        for s in range(len(chunks)):
            for w in range(NWIN):
                rw_ = int(call_rounds[s, w])
                if rw_ == 0:
                    continue
                cb = int(col0[s, w])
                seg = idx_flat[c][cb * 128:(cb + rw_) * 128]
                wseg = np.tile(seg.reshape(-1, 16).T, (8, 1))
                iw[:, 8 * cb:8 * (cb + rw_)] = wseg
        idxw_all.append(iw)

    x_own_all = [x_full[c * SHARD:(c + 1) * SHARD] for c in range(NCORES)]
    schedule = (R_tw, call_rounds, col0, TR, chunks)
    return x_full, x_own_all, idxw_all, ew_all, schedule, inv


def kernel(x, edge_weight_norm, cheb_coeffs, group_scale, group_bias,
           rms_weight, edge_index):
    x = np.asarray(x, np.float32)
    assert x.shape == (N, H)
    x_full, x_own_all, idxw_all, ew_all, schedule, inv = _prep(
        x, edge_weight_norm, edge_index)

    params = np.zeros((128, 40), np.float32)
    params[:, 32] = RMS_EPS
    cheb = np.asarray(cheb_coeffs, np.float32)      # [4, 6]
    params[:, :24] = cheb.reshape(1, 24)
    params[:, 24:28] = np.asarray(group_scale, np.float32).reshape(1, 4)
    params[:, 28:32] = np.asarray(group_bias, np.float32).reshape(1, 4)
    rmsw = np.tile(np.asarray(rms_weight, np.float32).reshape(1, H), (128, 1))

    repeat = int(os.environ.get("KERNEL_REPEAT", "1"))
    out_shards = _build_and_run(x_full, x_own_all, idxw_all, ew_all, params,
                                rmsw, schedule, repeat=repeat)
    out_full = out_shards.reshape(FULL, H)
    out = np.empty((N, H), np.float32)
    mask = inv >= 0
    out[inv[mask]] = out_full[mask]
    return out



# revision 4
# speedup vs baseline: 45.6428x; 45.6428x over previous
import os
import sys

sys.path.insert(0, '/opt/trn_rl_repo')
import numpy as np

NCORES = 8
N = 100000
H = 128
GROUPS = 4
GC = H // GROUPS
K = 5
TILES = 98
SHARD = TILES * 128            # 12544 padded rows per core
FULL = SHARD * NCORES          # 100352
WIN = 2 * SHARD                # 25088 rows per gather window (int16-safe)
NWIN = 4
CS = 20                        # max slots per gather call
TB = 14                        # tiles per stst/store block
RMS_EPS = 1.1920929e-07

_cache = {}


def _prep(x, edge_weight_norm, edge_index):
    src = np.asarray(edge_index[0]).astype(np.int64)
    dst = np.asarray(edge_index[1]).astype(np.int64)
    ew = np.asarray(edge_weight_norm, dtype=np.float32)
    E = src.shape[0]

    # window class of a node = v % NWIN; class-w nodes are dealt to cores
    # {2w, 2w+1} so their padded positions fill exactly window w.
    # Within a class, sort by per-window in-degree profile so that nodes in
    # the same tile (and matching tiles across cores) have near-identical
    # profiles -> per-(window,tile) edge counts are balanced across cores.
    prof = np.zeros((N, NWIN), np.int32)
    np.add.at(prof, (dst, src % NWIN), 1)

    perm_pos = np.empty(N, np.int64)
    inv = np.full(FULL, -1, np.int64)
    CLS_N = N // NWIN
    for w in range(NWIN):
        nodes = np.arange(N)[np.arange(N) % NWIN == w]
        kk = prof[nodes]
        order = np.lexsort((kk[:, 3], kk[:, 2], kk[:, 1], kk[:, 0]))
        sn = nodes[order]
        rank = np.arange(CLS_N)
        core = 2 * w + (rank % 2)
        pos = core * SHARD + rank // 2
        perm_pos[sn] = pos
        inv[pos] = sn

    x_full = np.zeros((FULL, H), np.float32)
    x_full[perm_pos] = np.asarray(x, np.float32)

    src_p = perm_pos[src]
    dst_p = perm_pos[dst]
    dst_core = dst_p // SHARD
    win_id = src_p // WIN
    win_rel = (src_p - win_id * WIN).astype(np.int64)
    dst_local = dst_p - dst_core * SHARD
    tile_id = dst_local // 128
    part_id = dst_local % 128

    # static schedule: per (w, t) group, slots = ceil(max_core_count / 128)
    n_cwt = np.zeros((NCORES, NWIN, TILES), np.int64)
    np.add.at(n_cwt, (dst_core, win_id, tile_id), 1)
    mx_wt = n_cwt.max(axis=0)
    slots_wt = -(-mx_wt // 128)
    slot0_wt = np.zeros((NWIN, TILES), np.int64)
    run = 0
    for w in range(NWIN):
        for t in range(TILES):
            slot0_wt[w, t] = run
            run += int(slots_wt[w, t])
    TR = int(run)

    # calls: greedy-pack groups of one window into chunks of <= CS slots.
    # each call: (w, s0, ns, [(t, goff, gns)]) with goff relative to s0.
    calls = []
    for w in range(NWIN):
        cur = []
        cs0 = None
        acc = 0
        for t in range(TILES):
            s = int(slots_wt[w, t])
            if s == 0:
                continue
            if cur and acc + s > CS:
                calls.append((w, cs0, acc, cur))
                cur = []
                acc = 0
            if not cur:
                cs0 = int(slot0_wt[w, t])
            cur.append((t, acc, s))
            acc += s
        if cur:
            calls.append((w, cs0, acc, cur))

    # first window with edges per tile (drain = copy there, add later)
    first_w = np.full(TILES, -1, np.int64)
    for w in range(NWIN):
        for t in range(TILES):
            if slots_wt[w, t] and first_w[t] < 0:
                first_w[t] = w
    empty_tiles = [t for t in range(TILES) if first_w[t] < 0]

    # rank of edge within its (core, w, t) group
    key = (dst_core * NWIN + win_id) * TILES + tile_id
    order = np.argsort(key, kind='stable')
    ks = key[order]
    starts = np.r_[0, np.flatnonzero(np.diff(ks)) + 1]
    group_len = np.diff(np.r_[starts, E])
    rank_sorted = np.arange(E) - np.repeat(starts, group_len)
    rank = np.empty(E, np.int64)
    rank[order] = rank_sorted

    lane_global = slot0_wt[win_id, tile_id] * 128 + rank

    idxw_all = []
    tgt_all = []
    ewl_all = []
    for c in range(NCORES):
        m = dst_core == c
        idx_flat = np.zeros(TR * 128, np.int16)
        idx_flat[lane_global[m]] = win_rel[m].astype(np.int16)
        # per-(lane, slot) scatter target part and edge weight; P matrices
        # are generated on-device as (iota == tgt) * ew
        tgt = np.zeros((128, TR), np.float32)
        ewl = np.zeros((128, TR), np.float32)
        tgt[lane_global[m] % 128, lane_global[m] // 128] = part_id[m]
        ewl[lane_global[m] % 128, lane_global[m] // 128] = ew[m]
        tgt_all.append(tgt)
        ewl_all.append(ewl)
        # wrapped idx layout per call (16-partition wrap, replicated x8)
        iw = np.zeros((128, 8 * TR), np.int16)
        for (w, s0, ns, _) in calls:
            seg = idx_flat[s0 * 128:(s0 + ns) * 128]
            wseg = np.tile(seg.reshape(-1, 16).T, (8, 1))
            iw[:, 8 * s0:8 * (s0 + ns)] = wseg
        idxw_all.append(iw)

    x_own_all = [x_full[c * SHARD:(c + 1) * SHARD] for c in range(NCORES)]
    sched = (TR, calls, first_w, empty_tiles)
    return x_full, x_own_all, idxw_all, tgt_all, ewl_all, sched, inv


def _build_and_run(x_full, x_own_all, idxw_all, tgt_all, ewl_all, params_all,
                   rmsw_all, sched, repeat=1):
    from concourse import bacc, bass, mybir, tile
    from concourse.bass_utils import run_bass_kernel_spmd

    (TR, calls, first_w, empty_tiles) = sched
    f32 = mybir.dt.float32
    i16 = mybir.dt.int16

    key = ("prog3", TR, repeat,
           tuple((w, s0, ns) for (w, s0, ns, _) in calls),
           tuple(first_w), tuple(empty_tiles))
    if key in _cache:
        nc = _cache[key]
    else:
        nc = bacc.Bacc("TRN2", target_bir_lowering=False, debug=False,
                       num_devices=NCORES)
        xf = nc.dram_tensor("x_full", [FULL, H], f32, kind="ExternalInput")
        xo = nc.dram_tensor("x_own", [SHARD, H], f32, kind="ExternalInput")
        idx_in = nc.dram_tensor("idx_in", [128, 8 * TR], i16,
                                kind="ExternalInput")
        tgt_in = nc.dram_tensor("tgt_in", [128, TR], f32, kind="ExternalInput")
        ewl_in = nc.dram_tensor("ewl_in", [128, TR], f32, kind="ExternalInput")
        iota_in = nc.dram_tensor("iota_in", [128, CS * H], f32,
                                 kind="ExternalInput")
        par_in = nc.dram_tensor("par_in", [128, 40], f32, kind="ExternalInput")
        rw_in = nc.dram_tensor("rw_in", [128, H], f32, kind="ExternalInput")
        out_ext = nc.dram_tensor("out", [SHARD, H], f32, kind="ExternalOutput")

        with tile.TileContext(nc) as tc:
            with (
                tc.tile_pool(name="dram", bufs=1, space="DRAM") as dram,
                tc.tile_pool(name="big", bufs=1) as big,
                tc.tile_pool(name="gpool", bufs=3) as gpool,
                tc.tile_pool(name="ppool", bufs=3) as ppool,
                tc.tile_pool(name="small", bufs=2) as small,
                tc.tile_pool(name="psum", bufs=8, space="PSUM") as psum,
            ):
                shard_b = [dram.tile([SHARD, H], f32, tag=f"sh{k}",
                                     name=f"sh{k}") for k in range(4)]
                full_b = [dram.tile([FULL, H], f32, tag=f"fl{k}",
                                    name=f"fl{k}", addr_space="Shared")
                          for k in range(4)]

                idx_sb = big.tile([128, 8 * TR], i16, tag="idx")
                tgt_sb = big.tile([128, TR], f32, tag="tgt")
                ewl_sb = big.tile([128, TR], f32, tag="ewl")
                iota_sb = big.tile([128, CS * H], f32, tag="iota")
                par_sb = big.tile([128, 40], f32, tag="par")
                rw_sb = big.tile([128, H], f32, tag="rw")
                nc.sync.dma_start(idx_sb[:], idx_in[:])
                nc.sync.dma_start(tgt_sb[:], tgt_in[:])
                nc.sync.dma_start(ewl_sb[:], ewl_in[:])
                nc.sync.dma_start(iota_sb[:], iota_in[:])
                nc.sync.dma_start(par_sb[:], par_in[:])
                nc.sync.dma_start(rw_sb[:], rw_in[:])
                iota3 = iota_sb[:].rearrange("p (s j) -> p s j", j=H)

                result = big.tile([128, SHARD], f32, tag="res")
                tcur = big.tile([128, SHARD], f32, tag="tcur")

                def res3(ap):  # [128, SHARD] -> [p, t, j]
                    return ap.rearrange("p (t j) -> p t j", j=H)

                def gview(ap, g):  # group-g strided view [p, t, 32]
                    return res3(ap)[:, :, g * GC:(g + 1) * GC]

                def cheb(g, k):
                    return par_sb[:, g * 6 + k:g * 6 + k + 1]

                for rep in range(repeat):
                    if rep > 0:
                        full_b = [dram.tile([FULL, H], f32, tag=f"fl{k}_{rep}",
                                            name=f"fl{k}_{rep}",
                                            addr_space="Shared")
                                  for k in range(4)]
                    # result = c0 (x) x_own
                    nc.sync.dma_start(
                        res3(result[:]),
                        xo[:].rearrange("(t p) j -> p t j", p=128))
                    for g in range(GROUPS):
                        nc.vector.tensor_mul(
                            gview(result[:], g), gview(result[:], g),
                            cheb(g, 0).to_broadcast([128, TILES, GC]))

                    for hop in range(1, K + 1):
                        src_tab = xf if hop == 1 else full_b[hop - 2]
                        for (w, s0, ns, cgroups) in calls:
                            gbuf = gpool.tile([128, CS, H], f32, tag="g")
                            ptile = ppool.tile([128, CS, H], f32, tag="p")
                            nidx = ns * 128
                            nc.gpsimd.dma_gather(
                                gbuf[:, :ns, :],
                                src_tab[w * WIN:(w + 1) * WIN, :],
                                idx_sb[:, 8 * s0:8 * (s0 + ns)],
                                nidx, nidx, H, single_packet=False,
                            )
                            # P[lane, s, d] = (d == tgt[lane,s]) * ew[lane,s]
                            nc.vector.tensor_tensor(
                                ptile[:, :ns, :], iota3[:, :ns, :],
                                tgt_sb[:, s0:s0 + ns]
                                .rearrange("p (s o) -> p s o", o=1)
                                .to_broadcast([128, ns, H]),
                                mybir.AluOpType.is_equal)
                            nc.vector.tensor_mul(
                                ptile[:, :ns, :], ptile[:, :ns, :],
                                ewl_sb[:, s0:s0 + ns]
                                .rearrange("p (s o) -> p s o", o=1)
                                .to_broadcast([128, ns, H]))
                            for (t, goff, gns) in cgroups:
                                pt = psum.tile([128, H], f32, tag="ps")
                                for kk in range(gns):
                                    nc.tensor.matmul(
                                        pt[:],
                                        ptile[:, goff + kk, :],
                                        gbuf[:, goff + kk, :],
                                        start=(kk == 0), stop=(kk == gns - 1))
                                dstv = tcur[:, t * H:(t + 1) * H]
                                if w == first_w[t]:
                                    nc.vector.tensor_copy(dstv, pt[:])
                                else:
                                    nc.vector.tensor_add(dstv, dstv, pt[:])
                        for t in empty_tiles:
                            nc.vector.memset(tcur[:, t * H:(t + 1) * H], 0.0)
                        # tcur = 2*tcur - T_prev2 (hops >= 2); store shard
                        for b0 in range(0, TILES, TB):
                            b1 = min(TILES, b0 + TB)
                            nt = b1 - b0
                            if hop >= 2:
                                tp2 = small.tile([128, TB, H], f32, tag="tp2")
                                pb = shard_b[hop - 3][:] if hop >= 3 else xo[:]
                                nc.sync.dma_start(
                                    tp2[:, :nt, :],
                                    pb[b0 * 128:b1 * 128, :]
                                    .rearrange("(t p) j -> p t j", p=128))
                                nc.vector.scalar_tensor_tensor(
                                    res3(tcur[:])[:, b0:b1, :],
                                    res3(tcur[:])[:, b0:b1, :], 2.0,
                                    tp2[:, :nt, :],
                                    mybir.AluOpType.mult,
                                    mybir.AluOpType.subtract)
                            if hop <= 4:
                                nc.sync.dma_start(
                                    shard_b[hop - 1]
                                    [b0 * 128:b1 * 128, :]
                                    .rearrange("(t p) j -> p t j", p=128),
                                    res3(tcur[:])[:, b0:b1, :])
                        # result += c_hop * tcur (tcur scaled in place after
                        # its DMA stores; safe, it is dead afterwards)
                        for g in range(GROUPS):
                            nc.vector.tensor_mul(
                                gview(tcur[:], g), gview(tcur[:], g),
                                cheb(g, hop).to_broadcast([128, TILES, GC]))
                        nc.vector.tensor_add(result[:], result[:], tcur[:])
                        if hop <= 4:
                            nc.gpsimd.collective_compute(
                                "AllGather", mybir.AluOpType.bypass,
                                replica_groups=[list(range(NCORES))],
                                ins=[shard_b[hop - 1][:].opt()],
                                outs=[full_b[hop - 1][:].opt()],
                            )

                    # epilogue: group scale/bias, RMSNorm, SiLU
                    for g in range(GROUPS):
                        nc.vector.tensor_mul(
                            gview(result[:], g), gview(result[:], g),
                            par_sb[:, 24 + g:25 + g]
                            .to_broadcast([128, TILES, GC]))
                        nc.vector.tensor_add(
                            gview(result[:], g), gview(result[:], g),
                            par_sb[:, 28 + g:29 + g]
                            .to_broadcast([128, TILES, GC]))
                    nc.vector.tensor_mul(tcur[:], result[:], result[:])
                    ssq = small.tile([128, TILES], f32, tag="ssq")
                    nc.vector.tensor_reduce(
                        ssq[:], res3(tcur[:]), mybir.AxisListType.X,
                        mybir.AluOpType.add)
                    rms = small.tile([128, TILES], f32, tag="rms")
                    sq = small.tile([128, TILES], f32, tag="sqr")
                    nc.scalar.activation(
                        sq[:], ssq[:], mybir.ActivationFunctionType.Sqrt,
                        bias=par_sb[:, 32:33], scale=1.0 / H)
                    nc.vector.reciprocal(rms[:], sq[:])
                    nc.vector.tensor_mul(
                        res3(result[:]), res3(result[:]),
                        rms[:].rearrange("p (t o) -> p t o", o=1).to_broadcast(
                            [128, TILES, H]))
                    nc.vector.tensor_mul(
                        res3(result[:]), res3(result[:]),
                        rw_sb[:].rearrange("p (o j) -> p o j", o=1)
                        .to_broadcast([128, TILES, H]))
                    nc.scalar.activation(
                        res3(tcur[:]), res3(result[:]),
                        mybir.ActivationFunctionType.Silu)
                    nc.sync.dma_start(
                        out_ext[:].rearrange("(t p) j -> p t j", p=128),
                        res3(tcur[:]))

        nc.compile()
        _cache[key] = nc

    iota = np.tile(np.arange(H, dtype=np.float32), (128, CS))
    in_maps = []
    for c in range(NCORES):
        in_maps.append({
            "x_full": x_full,
            "x_own": x_own_all[c],
            "idx_in": idxw_all[c],
            "tgt_in": tgt_all[c],
            "ewl_in": ewl_all[c],
            "iota_in": iota,
            "par_in": params_all,
            "rw_in": rmsw_all,
        })
    trace_kw = {}
    if os.environ.get("KERNEL_TRACE"):
        trace_kw = dict(trace=True, tmpdir=os.environ["KERNEL_TRACE"])
    res = run_bass_kernel_spmd(nc, in_maps, list(range(NCORES)), **trace_kw)
    if trace_kw:
        print("exec_time_ns:", res.exec_time_ns)
        _cache["exec_time_ns"] = res.exec_time_ns
    return np.stack([res.results[c]["out"] for c in range(NCORES)], axis=0)


def kernel(x, edge_weight_norm, cheb_coeffs, group_scale, group_bias,
           rms_weight, edge_index):
    x = np.asarray(x, np.float32)
    assert x.shape == (N, H)
    x_full, x_own_all, idxw_all, tgt_all, ewl_all, sched, inv = _prep(
        x, edge_weight_norm, edge_index)

    params = np.zeros((128, 40), np.float32)
    params[:, 32] = RMS_EPS
    cheb = np.asarray(cheb_coeffs, np.float32)      # [4, 6]
    params[:, :24] = cheb.reshape(1, 24)
    params[:, 24:28] = np.asarray(group_scale, np.float32).reshape(1, 4)
    params[:, 28:32] = np.asarray(group_bias, np.float32).reshape(1, 4)
    rmsw = np.tile(np.asarray(rms_weight, np.float32).reshape(1, H), (128, 1))

    repeat = int(os.environ.get("KERNEL_REPEAT", "1"))
    out_shards = _build_and_run(x_full, x_own_all, idxw_all, tgt_all, ewl_all,
                                params, rmsw, sched, repeat=repeat)
    out_full = out_shards.reshape(FULL, H)
    out = np.empty((N, H), np.float32)
    mask = inv >= 0
    out[inv[mask]] = out_full[mask]
    return out


# revision 5
# speedup vs baseline: 59.4848x; 1.3033x over previous
import os
import sys

sys.path.insert(0, '/opt/trn_rl_repo')
import numpy as np

NCORES = 8
N = 100000
H = 128
GROUPS = 4
GC = H // GROUPS
K = 5
TILES = 98
SHARD = TILES * 128            # 12544 padded rows per core
FULL = SHARD * NCORES          # 100352
WIN = 2 * SHARD                # 25088 rows per gather window (int16-safe)
NWIN = 4
CS = 14                        # max slots per gather call / cell
TB = 14                        # tiles per stst/store block
RMS_EPS = 1.1920929e-07

_cache = {}


def _prep(x, edge_weight_norm, edge_index):
    src = np.asarray(edge_index[0]).astype(np.int64)
    dst = np.asarray(edge_index[1]).astype(np.int64)
    ew = np.asarray(edge_weight_norm, dtype=np.float32)
    E = src.shape[0]

    # window class of a node = v % NWIN; class-w nodes are dealt to cores
    # {2w, 2w+1} so their padded positions fill exactly window w.
    # Within a class, sort by per-window in-degree profile so that nodes in
    # the same tile (and matching tiles across cores) have near-identical
    # profiles -> per-(window,tile) edge counts are balanced across cores.
    prof = np.zeros((N, NWIN), np.int32)
    np.add.at(prof, (dst, src % NWIN), 1)

    perm_pos = np.empty(N, np.int64)
    inv = np.full(FULL, -1, np.int64)
    CLS_N = N // NWIN
    for w in range(NWIN):
        nodes = np.arange(N)[np.arange(N) % NWIN == w]
        kk = prof[nodes]
        order = np.lexsort((kk[:, 3], kk[:, 2], kk[:, 1], kk[:, 0]))
        sn = nodes[order]
        rank = np.arange(CLS_N)
        core = 2 * w + (rank % 2)
        pos = core * SHARD + rank // 2
        perm_pos[sn] = pos
        inv[pos] = sn

    x_full = np.zeros((FULL, H), np.float32)
    x_full[perm_pos] = np.asarray(x, np.float32)

    src_p = perm_pos[src]
    dst_p = perm_pos[dst]
    dst_core = dst_p // SHARD
    win_id = src_p // WIN
    win_rel = (src_p - win_id * WIN).astype(np.int64)
    dst_local = dst_p - dst_core * SHARD
    tile_id = dst_local // 128
    part_id = dst_local % 128

    # static schedule: tiles get lane-granular spans span_wt = max_core_count;
    # consecutive tiles of one window merge into cells (DP-chosen) so ceil
    # rounding to 128-lane slots happens once per cell, not once per tile.
    n_cwt = np.zeros((NCORES, NWIN, TILES), np.int64)
    np.add.at(n_cwt, (dst_core, win_id, tile_id), 1)
    span_wt = n_cwt.max(axis=0)   # [NWIN, TILES] lanes per (w, t)

    # DP per window: partition tile sequence into cells, cell slot cap CS,
    # cost = ceil(cell_lanes / 128)
    cells = []   # (w, [tiles])
    for w in range(NWIN):
        ts = [t for t in range(TILES) if span_wt[w, t] > 0]
        nt = len(ts)
        INF = 1 << 40
        best = [INF] * (nt + 1)
        prev = [0] * (nt + 1)
        best[0] = 0
        for j in range(1, nt + 1):
            lanes = 0
            for i in range(j - 1, -1, -1):
                lanes += int(span_wt[w, ts[i]])
                slots = -(-lanes // 128)
                if slots > CS:
                    break
                c = best[i] + slots * 128 + 2  # small per-cell penalty
                if c < best[j]:
                    best[j] = c
                    prev[j] = i
        bnd = []
        j = nt
        while j > 0:
            bnd.append((prev[j], j))
            j = prev[j]
        for (i, j) in reversed(bnd):
            cells.append((w, ts[i:j]))

    # lane layout: per cell, tiles packed back-to-back (static boundaries),
    # cell start aligned to a slot boundary
    lane0_wt = np.zeros((NWIN, TILES), np.int64)
    cell_info = []   # (w, s0, ns, [(t, lane_a, lane_b)]) lanes rel to s0*128
    run_slots = 0
    for (w, ts) in cells:
        s0 = run_slots
        la = 0
        tl = []
        for t in ts:
            lane0_wt[w, t] = s0 * 128 + la
            tl.append((t, la, la + int(span_wt[w, t])))
            la += int(span_wt[w, t])
        ns = -(-la // 128)
        cell_info.append((w, s0, ns, tl))
        run_slots += ns
    TR = int(run_slots)

    # per cell: MM schedule. MM = (slot_rel, tile, mm_col); per tile the MMs
    # cover its overlapping slots in order (start on first, stop on last).
    # calls == cells (each cell is one gather call).
    calls = []
    nmm = 0
    for (w, s0, ns, tl) in cell_info:
        groups = []
        for (t, la, lb) in tl:
            mms = []
            for s in range(la // 128, -(-lb // 128)):
                mms.append((s, nmm, max(la, s * 128), min(lb, (s + 1) * 128)))
                nmm += 1
            groups.append((t, mms))
        calls.append((w, s0, ns, groups))
    NMM = nmm

    # first window with edges per tile (drain = copy there, add later)
    first_w = np.full(TILES, -1, np.int64)
    for w in range(NWIN):
        for t in range(TILES):
            if span_wt[w, t] and first_w[t] < 0:
                first_w[t] = w
    empty_tiles = [t for t in range(TILES) if first_w[t] < 0]

    # rank of edge within its (core, w, t) group
    key = (dst_core * NWIN + win_id) * TILES + tile_id
    order = np.argsort(key, kind='stable')
    ks = key[order]
    starts = np.r_[0, np.flatnonzero(np.diff(ks)) + 1]
    group_len = np.diff(np.r_[starts, E])
    rank_sorted = np.arange(E) - np.repeat(starts, group_len)
    rank = np.empty(E, np.int64)
    rank[order] = rank_sorted

    lane_global = lane0_wt[win_id, tile_id] + rank

    PMAX = max(sum(len(mms) for (_, mms) in groups)
               for (_, _, _, groups) in calls)

    idxw_all = []
    tgt_all = []
    ewl_all = []
    for c in range(NCORES):
        m = dst_core == c
        idx_flat = np.zeros(TR * 128, np.int16)
        idx_flat[lane_global[m]] = win_rel[m].astype(np.int16)
        # per-lane scatter target part / edge weight, sliced per MM column;
        # P matrices are generated on-device as (iota == tgt) * ew
        tgtL = np.zeros(TR * 128, np.float32)
        ewL = np.zeros(TR * 128, np.float32)
        tgtL[lane_global[m]] = part_id[m]
        ewL[lane_global[m]] = ew[m]
        tgt2 = np.zeros((128, NMM), np.float32)
        ewl2 = np.zeros((128, NMM), np.float32)
        for (w, s0, ns, groups) in calls:
            for (t, mms) in groups:
                for (s_rel, mcol, a, b) in mms:
                    p0 = a - s_rel * 128
                    glo = s0 * 128 + a
                    tgt2[p0:p0 + (b - a), mcol] = tgtL[glo:glo + (b - a)]
                    ewl2[p0:p0 + (b - a), mcol] = ewL[glo:glo + (b - a)]
        tgt_all.append(tgt2)
        ewl_all.append(ewl2)
        # wrapped idx layout per call (16-partition wrap, replicated x8)
        iw = np.zeros((128, 8 * TR), np.int16)
        for (w, s0, ns, _) in calls:
            seg = idx_flat[s0 * 128:(s0 + ns) * 128]
            wseg = np.tile(seg.reshape(-1, 16).T, (8, 1))
            iw[:, 8 * s0:8 * (s0 + ns)] = wseg
        idxw_all.append(iw)

    x_own_all = [x_full[c * SHARD:(c + 1) * SHARD] for c in range(NCORES)]
    sched = (TR, NMM, PMAX, calls, first_w, empty_tiles)
    return x_full, x_own_all, idxw_all, tgt_all, ewl_all, sched, inv


def _build_and_run(x_full, x_own_all, idxw_all, tgt_all, ewl_all, params_all,
                   rmsw_all, sched, repeat=1):
    from concourse import bacc, bass, mybir, tile
    from concourse.bass_utils import run_bass_kernel_spmd

    (TR, NMM, PMAX, calls, first_w, empty_tiles) = sched
    f32 = mybir.dt.float32
    i16 = mybir.dt.int16

    key = ("prog5", TR, NMM, PMAX, repeat,
           tuple((w, s0, ns) for (w, s0, ns, _) in calls),
           tuple(first_w), tuple(empty_tiles))
    if key in _cache:
        nc = _cache[key]
    else:
        nc = bacc.Bacc("TRN2", target_bir_lowering=False, debug=False,
                       num_devices=NCORES)
        xf = nc.dram_tensor("x_full", [FULL, H], f32, kind="ExternalInput")
        xo = nc.dram_tensor("x_own", [SHARD, H], f32, kind="ExternalInput")
        idx_in = nc.dram_tensor("idx_in", [128, 8 * TR], i16,
                                kind="ExternalInput")
        tgt_in = nc.dram_tensor("tgt_in", [128, NMM], f32,
                                kind="ExternalInput")
        ewl_in = nc.dram_tensor("ewl_in", [128, NMM], f32,
                                kind="ExternalInput")
        iota_in = nc.dram_tensor("iota_in", [128, PMAX * H], f32,
                                 kind="ExternalInput")
        par_in = nc.dram_tensor("par_in", [128, 40], f32, kind="ExternalInput")
        rw_in = nc.dram_tensor("rw_in", [128, H], f32, kind="ExternalInput")
        out_ext = nc.dram_tensor("out", [SHARD, H], f32, kind="ExternalOutput")

        with tile.TileContext(nc) as tc:
            with (
                tc.tile_pool(name="dram", bufs=1, space="DRAM") as dram,
                tc.tile_pool(name="big", bufs=1) as big,
                tc.tile_pool(name="gpool", bufs=3) as gpool,
                tc.tile_pool(name="ppool", bufs=2) as ppool,
                tc.tile_pool(name="small", bufs=2) as small,
                tc.tile_pool(name="psum", bufs=8, space="PSUM") as psum,
            ):
                shard_b = [dram.tile([SHARD, H], f32, tag=f"sh{k}",
                                     name=f"sh{k}") for k in range(4)]
                full_b = [dram.tile([FULL, H], f32, tag=f"fl{k}",
                                    name=f"fl{k}", addr_space="Shared")
                          for k in range(4)]

                idx_sb = big.tile([128, 8 * TR], i16, tag="idx")
                tgt_sb = big.tile([128, NMM], f32, tag="tgt")
                ewl_sb = big.tile([128, NMM], f32, tag="ewl")
                iota_sb = big.tile([128, PMAX * H], f32, tag="iota")
                par_sb = big.tile([128, 40], f32, tag="par")
                rw_sb = big.tile([128, H], f32, tag="rw")
                nc.sync.dma_start(idx_sb[:], idx_in[:])
                nc.sync.dma_start(tgt_sb[:], tgt_in[:])
                nc.sync.dma_start(ewl_sb[:], ewl_in[:])
                nc.sync.dma_start(iota_sb[:], iota_in[:])
                nc.sync.dma_start(par_sb[:], par_in[:])
                nc.sync.dma_start(rw_sb[:], rw_in[:])
                iota3 = iota_sb[:].rearrange("p (s j) -> p s j", j=H)

                result = big.tile([128, SHARD], f32, tag="res")
                tcur = big.tile([128, SHARD], f32, tag="tcur")

                def res3(ap):  # [128, SHARD] -> [p, t, j]
                    return ap.rearrange("p (t j) -> p t j", j=H)

                def gview(ap, g):  # group-g strided view [p, t, 32]
                    return res3(ap)[:, :, g * GC:(g + 1) * GC]

                def cheb(g, k):
                    return par_sb[:, g * 6 + k:g * 6 + k + 1]

                for rep in range(repeat):
                    if rep > 0:
                        full_b = [dram.tile([FULL, H], f32, tag=f"fl{k}_{rep}",
                                            name=f"fl{k}_{rep}",
                                            addr_space="Shared")
                                  for k in range(4)]
                    # result = c0 (x) x_own
                    nc.sync.dma_start(
                        res3(result[:]),
                        xo[:].rearrange("(t p) j -> p t j", p=128))
                    for g in range(GROUPS):
                        nc.vector.tensor_mul(
                            gview(result[:], g), gview(result[:], g),
                            cheb(g, 0).to_broadcast([128, TILES, GC]))

                    for hop in range(1, K + 1):
                        src_tab = xf if hop == 1 else full_b[hop - 2]
                        for (w, s0, ns, cgroups) in calls:
                            gbuf = gpool.tile([128, CS, H], f32, tag="g")
                            ptile = ppool.tile([128, PMAX, H], f32, tag="p")
                            nidx = ns * 128
                            nc.gpsimd.dma_gather(
                                gbuf[:, :ns, :],
                                src_tab[w * WIN:(w + 1) * WIN, :],
                                idx_sb[:, 8 * s0:8 * (s0 + ns)],
                                nidx, nidx, H, single_packet=False,
                            )
                            m0 = cgroups[0][1][0][1]
                            nm = sum(len(mms) for (_, mms) in cgroups)
                            # P[lane, m, d] = (d == tgt[lane,m]) * ew[lane,m]
                            nc.vector.tensor_tensor(
                                ptile[:, :nm, :], iota3[:, :nm, :],
                                tgt_sb[:, m0:m0 + nm]
                                .rearrange("p (s o) -> p s o", o=1)
                                .to_broadcast([128, nm, H]),
                                mybir.AluOpType.is_equal)
                            nc.vector.tensor_mul(
                                ptile[:, :nm, :], ptile[:, :nm, :],
                                ewl_sb[:, m0:m0 + nm]
                                .rearrange("p (s o) -> p s o", o=1)
                                .to_broadcast([128, nm, H]))
                            for (t, mms) in cgroups:
                                pt = psum.tile([128, H], f32, tag="ps")
                                for j, (s_rel, mcol, a, b) in enumerate(mms):
                                    nc.tensor.matmul(
                                        pt[:],
                                        ptile[:, mcol - m0, :],
                                        gbuf[:, s_rel, :],
                                        start=(j == 0),
                                        stop=(j == len(mms) - 1))
                                dstv = tcur[:, t * H:(t + 1) * H]
                                if w == first_w[t]:
                                    nc.vector.tensor_copy(dstv, pt[:])
                                else:
                                    nc.vector.tensor_add(dstv, dstv, pt[:])
                        for t in empty_tiles:
                            nc.vector.memset(tcur[:, t * H:(t + 1) * H], 0.0)
                        # tcur = 2*tcur - T_prev2 (hops >= 2); store shard
                        for b0 in range(0, TILES, TB):
                            b1 = min(TILES, b0 + TB)
                            nt = b1 - b0
                            if hop >= 2:
                                tp2 = small.tile([128, TB, H], f32, tag="tp2")
                                pb = shard_b[hop - 3][:] if hop >= 3 else xo[:]
                                nc.sync.dma_start(
                                    tp2[:, :nt, :],
                                    pb[b0 * 128:b1 * 128, :]
                                    .rearrange("(t p) j -> p t j", p=128))
                                nc.vector.scalar_tensor_tensor(
                                    res3(tcur[:])[:, b0:b1, :],
                                    res3(tcur[:])[:, b0:b1, :], 2.0,
                                    tp2[:, :nt, :],
                                    mybir.AluOpType.mult,
                                    mybir.AluOpType.subtract)
                            if hop <= 4:
                                nc.sync.dma_start(
                                    shard_b[hop - 1]
                                    [b0 * 128:b1 * 128, :]
                                    .rearrange("(t p) j -> p t j", p=128),
                                    res3(tcur[:])[:, b0:b1, :])
                        # result += c_hop * tcur (tcur scaled in place after
                        # its DMA stores; safe, it is dead afterwards)
                        for g in range(GROUPS):
                            nc.vector.tensor_mul(
                                gview(tcur[:], g), gview(tcur[:], g),
                                cheb(g, hop).to_broadcast([128, TILES, GC]))
                        nc.vector.tensor_add(result[:], result[:], tcur[:])
                        if hop <= 4:
                            nc.gpsimd.collective_compute(
                                "AllGather", mybir.AluOpType.bypass,
                                replica_groups=[list(range(NCORES))],
                                ins=[shard_b[hop - 1][:].opt()],
                                outs=[full_b[hop - 1][:].opt()],
                            )

                    # epilogue: group scale/bias, RMSNorm, SiLU
                    for g in range(GROUPS):
                        nc.vector.tensor_mul(
                            gview(result[:], g), gview(result[:], g),
                            par_sb[:, 24 + g:25 + g]
                            .to_broadcast([128, TILES, GC]))
                        nc.vector.tensor_add(
                            gview(result[:], g), gview(result[:], g),
                            par_sb[:, 28 + g:29 + g]
                            .to_broadcast([128, TILES, GC]))
                    nc.vector.tensor_mul(tcur[:], result[:], result[:])
                    ssq = small.tile([128, TILES], f32, tag="ssq")
                    nc.vector.tensor_reduce(
                        ssq[:], res3(tcur[:]), mybir.AxisListType.X,
                        mybir.AluOpType.add)
                    rms = small.tile([128, TILES], f32, tag="rms")
                    sq = small.tile([128, TILES], f32, tag="sqr")
                    nc.scalar.activation(
                        sq[:], ssq[:], mybir.ActivationFunctionType.Sqrt,
                        bias=par_sb[:, 32:33], scale=1.0 / H)
                    nc.vector.reciprocal(rms[:], sq[:])
                    nc.vector.tensor_mul(
                        res3(result[:]), res3(result[:]),
                        rms[:].rearrange("p (t o) -> p t o", o=1).to_broadcast(
                            [128, TILES, H]))
                    nc.vector.tensor_mul(
                        res3(result[:]), res3(result[:]),
                        rw_sb[:].rearrange("p (o j) -> p o j", o=1)
                        .to_broadcast([128, TILES, H]))
                    nc.scalar.activation(
                        res3(tcur[:]), res3(result[:]),
                        mybir.ActivationFunctionType.Silu)
                    nc.sync.dma_start(
                        out_ext[:].rearrange("(t p) j -> p t j", p=128),
                        res3(tcur[:]))

        nc.compile()
        _cache[key] = nc

    iota = np.tile(np.arange(H, dtype=np.float32), (128, sched[2]))
    in_maps = []
    for c in range(NCORES):
        in_maps.append({
            "x_full": x_full,
            "x_own": x_own_all[c],
            "idx_in": idxw_all[c],
            "tgt_in": tgt_all[c],
            "ewl_in": ewl_all[c],
            "iota_in": iota,
            "par_in": params_all,
            "rw_in": rmsw_all,
        })
    trace_kw = {}
    if os.environ.get("KERNEL_TRACE"):
        trace_kw = dict(trace=True, tmpdir=os.environ["KERNEL_TRACE"])
    res = run_bass_kernel_spmd(nc, in_maps, list(range(NCORES)), **trace_kw)
    if trace_kw:
        print("exec_time_ns:", res.exec_time_ns)
        _cache["exec_time_ns"] = res.exec_time_ns
    return np.stack([res.results[c]["out"] for c in range(NCORES)], axis=0)


def kernel(x, edge_weight_norm, cheb_coeffs, group_scale, group_bias,
           rms_weight, edge_index):
    x = np.asarray(x, np.float32)
    assert x.shape == (N, H)
    x_full, x_own_all, idxw_all, tgt_all, ewl_all, sched, inv = _prep(
        x, edge_weight_norm, edge_index)

    params = np.zeros((128, 40), np.float32)
    params[:, 32] = RMS_EPS
    cheb = np.asarray(cheb_coeffs, np.float32)      # [4, 6]
    params[:, :24] = cheb.reshape(1, 24)
    params[:, 24:28] = np.asarray(group_scale, np.float32).reshape(1, 4)
    params[:, 28:32] = np.asarray(group_bias, np.float32).reshape(1, 4)
    rmsw = np.tile(np.asarray(rms_weight, np.float32).reshape(1, H), (128, 1))

    repeat = int(os.environ.get("KERNEL_REPEAT", "1"))
    out_shards = _build_and_run(x_full, x_own_all, idxw_all, tgt_all, ewl_all,
                                params, rmsw, sched, repeat=repeat)
    out_full = out_shards.reshape(FULL, H)
    out = np.empty((N, H), np.float32)
    mask = inv >= 0
    out[inv[mask]] = out_full[mask]
    return out
